# revision 13
# baseline (speedup 1.0000x reference)
"""Trainium2 Bass kernel v4 for nn_AttentionEncoder (GNN message passing).

Computation per (b, n):
    scores[k] = <x[b,n,:], neighbor[b,n,k,:]> / sqrt(D)        (K=32, D=128)
    attn      = softmax(scores)
    out[b,n]  = x[b,n] + sum_k attn[k] * neighbor[b,n,k]

Sharding: batch B=8 -> one batch per NeuronCore (8 cores), no communication.

Per-core design, per tile of P=128 nodes (nodes on partitions). Engine
assignment keeps every engine under the ~6.5us/tile DMA time of the 2MB
neighbor tile (neuronx-cc rejects fused scalar_tensor_tensor / scan on the
Pool engine, so GPSIMD only runs tensor_tensor / tensor_scalar):

  - scores  : k >= SCORE_GPS: fused DVE scalar_tensor_tensor per k
              (sp=(nb_k*1)*x, accum_out=s_k).
              k < SCORE_GPS: one bulk GPSIMD tensor_tensor product +
              one DVE tensor_reduce over those k.
  - softmax : ScalarE activation Exp(scale*s) with accum_out Z;
              rz = 1/Z on DVE. Max-subtraction skipped (scores ~ N(0,1)).
  - weighted products wp16[:,k,:] = e_k * nb[:,k,:] in bf16, split per k:
              WP_SCALAR k's on ScalarE (activation Copy, scale=e_k),
              WP_GPS k's on GPSIMD tensor_scalar, rest on DVE tensor_scalar.
  - k-reduce on PE: ps += I^T @ wp16_k, 32 accumulating bf16 matmuls with
              CONSTANT identity weights (one ldweights, no diag tiles).
  - output  : one DVE scalar_tensor_tensor: out = ps * rz + x
              (residual and softmax normalization in fp32).
"""

import numpy as np
from contextlib import ExitStack

import concourse.bass as bass
import concourse.tile as tile
from concourse import bacc, mybir
from concourse._compat import with_exitstack

B = 8
N = 10000
K = 32
D = 128
P = 128
SCALE = 1.0 / float(np.sqrt(np.float32(D)))
TG = 2  # node-tiles per DMA batch

# engine-balance knobs
SCORE_GPS = 0  # score k's computed as GPSIMD bulk product + DVE reduce
SCORE_DVE_BULK = False  # True: bulk tensor_tensor+reduce for the DVE score chunk
NB_PAIR = False  # load neighbor tiles two-at-a-time per dma_start
WP_SCALAR = 24  # weighted products on ScalarE
WP_GPS = 0  # weighted products on GPSIMD
WP_DVE = K - WP_SCALAR - WP_GPS  # rest on DVE
ABLATE = None  # bench-only probes: "noscore" | "nowp" | "dmaonly"
X_SCALAR = False  # issue x loads on the scalar queue instead of sync
NB_ALT = False  # alternate nb loads between sync and scalar queues
WP_GPS_BULK = 0  # trailing k's produced by ONE bulk GPSIMD tensor_tensor


def _wp_assignment():
    """Interleave the per-k weighted-product engine assignment so the PE
    consumption chain (k = 0..K-1, in order) is fed by all engines
    concurrently rather than in per-engine blocks."""
    quota = {"S": WP_SCALAR, "G": WP_GPS, "D": WP_DVE}
    total = float(K - WP_GPS_BULK)
    out = []
    done = {e: 0 for e in quota}
    for k in range(K - WP_GPS_BULK):
        # pick the engine furthest behind its proportional schedule
        best, gap = None, None
        for e, q in quota.items():
            if q == 0:
                continue
            g = done[e] - q * (k / total)
            if gap is None or g < gap:
                best, gap = e, g
        out.append(best)
        done[best] += 1
    return out


WP_ENGINE = _wp_assignment()

BENCH_REPS = 800  # on-device kernel repetitions per bench dispatch

F32 = mybir.dt.float32
BF16 = mybir.dt.bfloat16


@with_exitstack
def _attn_kernel(ctx: ExitStack, tc: "tile.TileContext", out_d, x_d, nb_d, n_nodes):
    nc = tc.nc

    singles = ctx.enter_context(tc.tile_pool(name="singles", bufs=1))
    nb_pool = ctx.enter_context(tc.tile_pool(name="nb", bufs=2 if NB_PAIR else 4))
    tail_pool = ctx.enter_context(tc.tile_pool(name="nbtail", bufs=1))
    x_pool = ctx.enter_context(tc.tile_pool(name="xp", bufs=6))
    out_pool = ctx.enter_context(tc.tile_pool(name="outp", bufs=4))
    wp_pool = ctx.enter_context(tc.tile_pool(name="wp", bufs=3))
    scr_pool = ctx.enter_context(tc.tile_pool(name="scr", bufs=3))
    small = ctx.enter_context(tc.tile_pool(name="small", bufs=16))
    psum_pool = ctx.enter_context(tc.tile_pool(name="psum", bufs=6, space="PSUM"))

    # One-time bf16 identity (constant PE weights).
    ident = singles.tile([P, P], BF16)
    nc.vector.memset(ident, 1.0)
    nc.gpsimd.affine_select(
        out=ident,
        in_=ident,
        pattern=[[-1, P]],
        compare_op=mybir.AluOpType.is_equal,
        fill=0.0,
        base=0,
        channel_multiplier=1,
    )

    ntiles = (n_nodes + P - 1) // P

    loaded = {}  # t -> (nb_t, x_t, rows)

    def dma_in(t):
        if t in loaded:
            return
        base = t * P
        if NB_PAIR and t + 1 < ntiles and (t + 2) * P <= n_nodes:
            # two full tiles with one dma_start each for nb and x
            nb2 = nb_pool.tile([P, 2, K, D], F32)
            x2 = x_pool.tile([P, 2, D], F32)
            nc.sync.dma_start(
                out=nb2,
                in_=nb_d[base : base + 2 * P].rearrange("(tg p) k d -> p tg k d", p=P),
            )
            nc.sync.dma_start(
                out=x2,
                in_=x_d[base : base + 2 * P].rearrange("(tg p) d -> p tg d", p=P),
            )
            loaded[t] = (nb2[:, 0], x2[:, 0], P)
            loaded[t + 1] = (nb2[:, 1], x2[:, 1], P)
            return
        rows = min(P, n_nodes - base)
        pool = tail_pool if NB_PAIR else nb_pool
        nb_t = pool.tile([P, K, D], F32)
        x_t = x_pool.tile([P, D], F32)
        alt_engine = nc.gpsimd if NB_ALT == "gpsimd" else nc.scalar
        nb_q = alt_engine if (NB_ALT and t % 2 == 1) else nc.sync
        x_q = nc.scalar if X_SCALAR else nc.sync
        nb_q.dma_start(out=nb_t[:rows], in_=nb_d[base : base + rows])
        x_q.dma_start(out=x_t[:rows], in_=x_d[base : base + rows])
        loaded[t] = (nb_t, x_t, rows)

    def stage_a(t):
        """Front half of tile t: scores, exp. Returns state dict."""
        nb_t, x_t, rows = loaded.pop(t)

        if ABLATE in ("noscore", "dmaonly"):
            e_t = small.tile([P, K], F32)
            z_t = small.tile([P, 1], F32)
            nc.vector.memset(e_t[:rows], 1.0)
            nc.vector.memset(z_t[:rows], float(K))
            return dict(t=t, rows=rows, nb_t=nb_t, x_t=x_t, e_t=e_t, z_t=z_t)

        # --- scores: s[:, k] = sum_d nb[:,k,d] * x[:,d] ----------------------
        s_t = small.tile([P, K], F32)
        if SCORE_GPS > 0:
            # GPS bulk product for k < SCORE_GPS (DVE reduces it later, after
            # its own per-k chunk, so the two engines run concurrently).
            sp_g = scr_pool.tile([P, SCORE_GPS, D], F32)
            xa = x_t[:rows]
            x_bc = bass.AP(
                tensor=xa.tensor,
                offset=xa.offset,
                ap=[xa.ap[0], [0, SCORE_GPS], xa.ap[-1]],
            )
            nc.gpsimd.tensor_tensor(
                out=sp_g[:rows],
                in0=nb_t[:rows, :SCORE_GPS],
                in1=x_bc,
                op=mybir.AluOpType.mult,
            )
        if SCORE_DVE_BULK:
            nd = K - SCORE_GPS
            sp_b = scr_pool.tile([P, nd, D], F32)
            xa2 = x_t[:rows]
            x_bc2 = bass.AP(
                tensor=xa2.tensor,
                offset=xa2.offset,
                ap=[xa2.ap[0], [0, nd], xa2.ap[-1]],
            )
            nc.vector.tensor_tensor(
                out=sp_b[:rows],
                in0=nb_t[:rows, SCORE_GPS:],
                in1=x_bc2,
                op=mybir.AluOpType.mult,
            )
            nc.vector.tensor_reduce(
                out=s_t[:rows, SCORE_GPS:],
                in_=sp_b[:rows],
                axis=mybir.AxisListType.X,
                op=mybir.AluOpType.add,
            )
        else:
            sp_d = small.tile([P, D], F32)  # DVE product dump (reused per k)
            for k in range(SCORE_GPS, K):
                nc.vector.scalar_tensor_tensor(
                    out=sp_d[:rows],
                    in0=nb_t[:rows, k],
                    scalar=1.0,
                    in1=x_t[:rows],
                    op0=mybir.AluOpType.mult,
                    op1=mybir.AluOpType.mult,
                    accum_out=s_t[:rows, k : k + 1],
                )
        if SCORE_GPS > 0:
            nc.vector.tensor_reduce(
                out=s_t[:rows, :SCORE_GPS],
                in_=sp_g[:rows],
                axis=mybir.AxisListType.X,
                op=mybir.AluOpType.add,
            )

        # --- E = exp(s*SCALE), Z = sum_k E -----------------------------------
        e_t = small.tile([P, K], F32)
        z_t = small.tile([P, 1], F32)
        nc.scalar.activation(
            out=e_t[:rows],
            in_=s_t[:rows],
            func=mybir.ActivationFunctionType.Exp,
            scale=SCALE,
            accum_out=z_t[:rows],
        )
        return dict(t=t, rows=rows, nb_t=nb_t, x_t=x_t, e_t=e_t, z_t=z_t)

    def stage_b_scalar(st):
        """ScalarE/GPSIMD share of tile t's weighted products (emitted one
        iteration later, BEFORE the next tile's scores, so ScalarE starts the
        moment e_t exists)."""
        t, rows = st["t"], st["rows"]
        nb_t, e_t = st["nb_t"], st["e_t"]
        if ABLATE in ("nowp", "dmaonly"):
            return
        wp16_t = wp_pool.tile([P, K, D], BF16)
        st["wp16_t"] = wp16_t
        if WP_GPS_BULK > 0:
            g = WP_GPS_BULK
            ea = e_t[:rows, K - g :]
            e_bc = bass.AP(
                tensor=ea.tensor,
                offset=ea.offset,
                ap=[ea.ap[0], ea.ap[-1], [0, D]],
            )
            nc.gpsimd.tensor_tensor(
                out=wp16_t[:rows, K - g :],
                in0=nb_t[:rows, K - g :],
                in1=e_bc,
                op=mybir.AluOpType.mult,
            )
        for k in range(K - WP_GPS_BULK):
            which = WP_ENGINE[k]
            if which == "S":
                nc.scalar.activation(
                    out=wp16_t[:rows, k],
                    in_=nb_t[:rows, k],
                    func=mybir.ActivationFunctionType.Copy,
                    scale=e_t[:rows, k : k + 1],
                )
            elif which == "G":
                nc.gpsimd.tensor_scalar_mul(
                    wp16_t[:rows, k],
                    in0=nb_t[:rows, k],
                    scalar1=e_t[:rows, k : k + 1],
                )

    def stage_b_dve(st):
        """DVE share of tile t's back half + the PE chain (emitted AFTER the
        next tile's scores, so the DVE never waits for exp/PE mid-stream)."""
        t, rows = st["t"], st["rows"]
        nb_t, e_t, z_t = st["nb_t"], st["e_t"], st["z_t"]

        rz_t = small.tile([P, 1], F32)
        nc.vector.reciprocal(out=rz_t[:rows], in_=z_t[:rows])
        st["rz_t"] = rz_t

        if ABLATE in ("nowp", "dmaonly"):
            st["out_ps"] = None
            return

        wp16_t = st["wp16_t"]
        for k in range(K - WP_GPS_BULK):
            if WP_ENGINE[k] == "D":
                nc.vector.tensor_scalar_mul(
                    wp16_t[:rows, k],
                    in0=nb_t[:rows, k],
                    scalar1=e_t[:rows, k : k + 1],
                )

        # --- k-reduction on PE: ps += I^T @ wp16_k (constant weights) --------
        out_ps = psum_pool.tile([P, D], F32)
        for k in range(K):
            nc.tensor.matmul(
                out_ps[:rows],
                lhsT=ident[:rows, :rows],
                rhs=wp16_t[:rows, k],
                start=(k == 0),
                stop=(k == K - 1),
            )
        st["out_ps"] = out_ps

    def stage_c(st):
        """Final: out = ps * rz + x. Emitted after the next tile's scores so
        the DVE never stalls on the PE chain."""
        t, rows = st["t"], st["rows"]
        out_t = out_pool.tile([P, D], F32)
        src_ps = st["out_ps"] if st["out_ps"] is not None else st["x_t"]
        nc.vector.scalar_tensor_tensor(
            out=out_t[:rows],
            in0=src_ps[:rows],
            scalar=st["rz_t"][:rows],
            in1=st["x_t"][:rows],
            op0=mybir.AluOpType.mult,
            op1=mybir.AluOpType.add,
        )
        st["out_t"] = out_t

    def stage_d(st):
        """DMA the finished tile out, one iteration after stage_c, so the
        issuing queue never waits on the final stt."""
        t, rows = st["t"], st["rows"]
        base = t * P
        nc.scalar.dma_start(out=out_d[base : base + rows], in_=st["out_t"][:rows])

    PF = 2  # DMA lookahead (tiles)
    pipe = []  # tiles awaiting (stage_b_dve, stage_c, stage_d)
    for t in range(ntiles):
        if t == 0:
            for d in range(min(PF + 1, ntiles)):
                dma_in(d)
        elif t + PF < ntiles:
            dma_in(t + PF)
        if len(pipe) >= 1:
            stage_b_scalar(pipe[-1])
        st = stage_a(t)
        if len(pipe) >= 1:
            stage_b_dve(pipe[-1])
        if len(pipe) >= 2:
            stage_c(pipe[-2])
        if len(pipe) >= 3:
            stage_d(pipe[-3])
            pipe.pop(0)
        pipe.append(st)
    # drain
    if len(pipe) >= 1:
        stage_b_scalar(pipe[-1])
        stage_b_dve(pipe[-1])
    if len(pipe) >= 2:
        stage_c(pipe[-2])
    if len(pipe) >= 3:
        stage_d(pipe[-3])
    stage_c(pipe[-1])
    if len(pipe) >= 2:
        stage_d(pipe[-2])
    stage_d(pipe[-1])


def _build(n_nodes: int = N, reps: int = 1):
    nc = bacc.Bacc(
        "TRN2",
        target_bir_lowering=False,
        debug=False,
        enable_asserts=False,
        num_devices=B,
    )
    x_d = nc.dram_tensor("x", [n_nodes, D], F32, kind="ExternalInput").ap()
    nb_d = nc.dram_tensor("neighbor", [n_nodes, K, D], F32, kind="ExternalInput").ap()
    out_d = nc.dram_tensor("out", [n_nodes, D], F32, kind="ExternalOutput").ap()
    with tile.TileContext(nc) as tc:
        if reps > 1:
            with tc.For_i(0, reps):
                _attn_kernel(tc, out_d, x_d, nb_d, n_nodes)
        else:
            _attn_kernel(tc, out_d, x_d, nb_d, n_nodes)
    nc.compile()
    return nc


_NC = None


def _get_nc():
    global _NC
    if _NC is None:
        _NC = _build(N)
    return _NC


def _run(x, neighbor, **spmd_kwargs):
    from concourse.bass_utils import run_bass_kernel_spmd

    nc = _get_nc()
    in_maps = [
        {
            "x": np.ascontiguousarray(np.asarray(x[b], dtype=np.float32)),
            "neighbor": np.ascontiguousarray(np.asarray(neighbor[b], dtype=np.float32)),
        }
        for b in range(B)
    ]
    res = run_bass_kernel_spmd(nc, in_maps, core_ids=list(range(B)), **spmd_kwargs)
    out = np.stack([r["out"] for r in res.results], axis=0)
    return out, res


def kernel(x, neighbor):
    out, _ = _run(x, neighbor)
    return out


def bench(x, neighbor, iters: int = 3, warmup: int = 1, reps: int = BENCH_REPS):
    """Time repeated on-device executions of the compiled kernel.

    The kernel body is wrapped in a hardware For_i loop executing `reps`
    times per NEFF dispatch, so the per-dispatch host/tunnel overhead is
    amortized away and the figure reflects steady-state device throughput.
    Returns (out, secs_per_kernel_iteration).
    """
    import time

    import jax
    from jax.sharding import Mesh, PartitionSpec, NamedSharding
    from jax.experimental.shard_map import shard_map

    import concourse.mybir as mybir_
    from concourse import bass2jax as b2j

    nc = _build(N, reps=reps)
    b2j.install_neuronx_cc_hook()

    partition_name = nc.partition_id_tensor.name if nc.partition_id_tensor else None
    in_names, out_names, out_avals = [], [], []
    for alloc in nc.m.functions[0].allocations:
        if not isinstance(alloc, mybir_.MemoryLocationSet):
            continue
        name = alloc.memorylocations[0].name
        if alloc.kind == "ExternalInput":
            if name != partition_name:
                in_names.append(name)
        elif alloc.kind == "ExternalOutput":
            out_names.append(name)
            out_avals.append(
                jax.core.ShapedArray(tuple(alloc.tensor_shape), mybir_.dt.np(alloc.dtype))
            )
    n_params = len(in_names)
    all_in_names = in_names + out_names
    if partition_name is not None:
        all_in_names = all_in_names + [partition_name]

    def _body(*args):
        operands = list(args)
        if partition_name is not None:
            operands.append(b2j.partition_id_tensor())
        outs = b2j._bass_exec_p.bind(
            *operands,
            out_avals=tuple(out_avals),
            in_names=tuple(all_in_names),
            out_names=tuple(out_names),
            lowering_input_output_aliases=(),
            sim_require_finite=True,
            sim_require_nnan=True,
            nc=nc,
        )
        return tuple(outs)

    devices = jax.devices()[:B]
    mesh = Mesh(np.asarray(devices), ("core",))
    spec = PartitionSpec("core")
    sharded = jax.jit(
        shard_map(
            _body,
            mesh=mesh,
            in_specs=(spec,) * (n_params + len(out_names)),
            out_specs=(spec,) * len(out_names),
            check_rep=False,
        ),
        keep_unused=True,
    )

    name_to_arr = {
        "x": np.ascontiguousarray(np.asarray(x, dtype=np.float32)).reshape(B * N, D),
        "neighbor": np.ascontiguousarray(np.asarray(neighbor, dtype=np.float32)).reshape(
            B * N, K, D
        ),
    }
    sh = NamedSharding(mesh, spec)
    dev_ins = [jax.device_put(name_to_arr[n], sh) for n in in_names]
    dev_zeros = [
        jax.device_put(np.zeros((B * a.shape[0], *a.shape[1:]), a.dtype), sh)
        for a in out_avals
    ]

    for _ in range(warmup):
        outs = sharded(*dev_ins, *dev_zeros)
        jax.block_until_ready(outs)
    t0 = time.perf_counter()
    for _ in range(iters):
        outs = sharded(*dev_ins, *dev_zeros)
    jax.block_until_ready(outs)
    t1 = time.perf_counter()

    out = np.asarray(outs[0]).reshape(B, N, D)
    return out, (t1 - t0) / (iters * reps)
